# revision 41
# baseline (speedup 1.0000x reference)
"""TransformerConv GNN (3 layers) on 8 Trainium2 NeuronCores.

Sharding: nodes split 3750/core (padded to 3840 = 30 tiles of 128).
Edges assigned to the core owning their dst node, grouped by 128-node
dst windows. Per layer:
  node phase: batched LN1 stats (one sqrt table load), kv projection
    sweep (bf16 matmuls, fused k|v weights) -> kv_bounce HBM; AllGather;
    q|skip projection sweep overlaps the AllGather.
  edge phase: dma_gather of kv[src] (4 SWDGE queues, 4-deep pipeline);
    onehot tiles resident in SBUF (scatter lhsT); transposed onehot
    loaded per batch via DMA-transpose (gather lhsT); q gathered on the
    PE via onehot matmul from SBUF-resident q; edge-attr projection on
    PE accumulated with gathered k|v in PSUM; attention alpha/exp/
    message on DVE+ACT; segment softmax accumulated per dst window via
    one-hot matmuls into PSUM.
  FFN phase: two sweeps (gelu sweep, then batched-LN2 + elu sweep) to
    avoid ACT table thrash; h updated in place.
Output head node-local; host reassembles shards.
"""
import contextlib
import math
import numpy as np

import concourse.bass as bass
import concourse.bacc as bacc
import concourse.tile as tile
from concourse import mybir, library_config
from concourse.bass_utils import run_bass_kernel_spmd

# problem dims
N, E, F, D, H, C, ED, L = 30000, 300000, 64, 128, 8, 16, 16, 3
NCORES = 8
NL = N // NCORES          # 3750 real nodes per core
NT = 30                   # node tiles per core
NLP = NT * 128            # 3840 padded nodes per core
KVROWS = NCORES * NLP     # kv table rows (global)
P = 128
G = 8                     # edge tiles per gather batch (dma_gather fails >1024 idxs/call)
B = 4                     # edge tiles per DVE op group

fp32 = mybir.dt.float32
bf16 = mybir.dt.bfloat16
i16 = mybir.dt.int16

AF = mybir.ActivationFunctionType
OP = mybir.AluOpType
AX = mybir.AxisListType


def _bcast3(ap, reps):
    """[P, k] AP -> [P, k, reps] with 0-stride last dim."""
    return bass.AP(tensor=ap.tensor, offset=ap.offset,
                   ap=[ap.ap[0], ap.ap[1], [0, reps]])


def _bcast4(ap, reps):
    """[P, b, k] AP -> [P, b, k, reps] with 0-stride last dim."""
    return bass.AP(tensor=ap.tensor, offset=ap.offset,
                   ap=[ap.ap[0], ap.ap[1], ap.ap[2], [0, reps]])


def build(tiles_per_window, skip_bias, debug=False):
    """Build the Bass program. tiles_per_window: NT ints, same per core."""
    import os
    nqueues = int(os.environ.get("K_Q", "4"))
    ilv = bool(int(os.environ.get("K_ILV", "0")))
    assert skip_bias, "non-zero bias path not implemented"
    tot_tiles = sum(tiles_per_window)
    tot_e = tot_tiles * 128
    nbatch = math.ceil(tot_tiles / G)

    tile_win, win_first, win_last = [], [], []
    for w, tw in enumerate(tiles_per_window):
        for i in range(tw):
            tile_win.append(w)
            win_first.append(i == 0)
            win_last.append(i == tw - 1)

    nc = bacc.Bacc("TRN2", target_bir_lowering=False, debug=False,
                   num_devices=NCORES, num_swdge_queues=nqueues)

    # ---------------- DRAM tensors ----------------
    x_in = nc.dram_tensor("x_shard", [NLP, F], fp32, kind="ExternalInput").ap()
    idx_src_d = nc.dram_tensor("idx_src", [P, tot_e // 16], i16,
                               kind="ExternalInput").ap()
    idx_dst_d = nc.dram_tensor("idx_dst", [P, tot_e // 16], i16,
                               kind="ExternalInput").ap()
    oh_d = nc.dram_tensor("onehot", [tot_e, P], bf16, kind="ExternalInput").ap()
    ea_d = nc.dram_tensor("ea_t", [ED, tot_e], bf16, kind="ExternalInput").ap()
    wkv_d = nc.dram_tensor("wkvT", [L, D, 2 * D], bf16,
                           kind="ExternalInput").ap()
    wqs_d = nc.dram_tensor("wqsT", [L, D, 2 * D], bf16,
                           kind="ExternalInput").ap()
    w1_d = nc.dram_tensor("w1T", [L, D, D], bf16, kind="ExternalInput").ap()
    w2_d = nc.dram_tensor("w2T", [L, D, D], bf16, kind="ExternalInput").ap()
    ewd_d = nc.dram_tensor("ewdT", [L, ED, 2 * D], bf16,
                           kind="ExternalInput").ap()
    w0_d = nc.dram_tensor("w0T", [F, D], fp32, kind="ExternalInput").ap()
    id_d = nc.dram_tensor("ident", [P, P], fp32, kind="ExternalInput").ap()
    wl_d = nc.dram_tensor("wlT", [D, 4], bf16, kind="ExternalInput").ap()
    out_d = nc.dram_tensor("out", [NLP, 4], fp32, kind="ExternalOutput").ap()

    kv_bounce = nc.dram_tensor("kv_bounce", [NLP, 2 * D], bf16).ap()
    kv_full = nc.dram_tensor("kv_full", [KVROWS, 2 * D], bf16,
                             addr_space="Shared").ap()
    q_tab = nc.dram_tensor("q_tab", [NLP, D], bf16).ap()

    dbg = {}
    if debug:
        for nm, sh, dt in [("dbg_h0", [NLP, D], fp32),
                           ("dbg_kv", [NLP, 2 * D], bf16),
                           ("dbg_q", [NLP, D], bf16),
                           ("dbg_skip", [NLP, D], bf16),
                           ("dbg_hc", [NLP, D], bf16),
                           ("dbg_h1", [NLP, D], fp32),
                           ("dbg_h2", [NLP, D], fp32),
                           ("dbg_qg", [tot_e, D], bf16),
                           ("dbg_pk", [tot_e, D + 8], bf16)]:
            dbg[nm] = nc.dram_tensor(nm, sh, dt, kind="ExternalOutput").ap()

    eps = 1e-5

    with tile.TileContext(nc) as tc:
        nc.gpsimd.load_library(library_config.mlp)
        with contextlib.ExitStack() as ctx:
            const = ctx.enter_context(tc.tile_pool(name="const", bufs=1))
            nodes = ctx.enter_context(tc.tile_pool(name="nodes", bufs=1))
            wpool = ctx.enter_context(tc.tile_pool(name="wpool", bufs=2))
            ntmp = ctx.enter_context(tc.tile_pool(name="ntmp", bufs=3))
            nsm = ctx.enter_context(tc.tile_pool(name="nsm", bufs=4))
            stat = ctx.enter_context(tc.tile_pool(name="stat", bufs=2))
            gbuf = ctx.enter_context(tc.tile_pool(name="gbuf", bufs=4))
            ebuf = ctx.enter_context(tc.tile_pool(name="ebuf", bufs=3))

            # constants
            id32 = const.tile([P, P], fp32, tag="id32")
            nc.sync.dma_start(out=id32[:], in_=id_d[:, :])
            id16 = const.tile([P, P], bf16, tag="id16")
            nc.vector.tensor_copy(out=id16[:], in_=id32[:])
            eps_t = const.tile([P, 1], fp32, tag="eps")
            nc.vector.memset(eps_t[:], eps)

            idx_src = const.tile([P, tot_e // 16], i16, tag="isrc")
            nc.sync.dma_start(out=idx_src[:], in_=idx_src_d[:, :])
            idx_dst = const.tile([P, tot_e // 16], i16, tag="idst")
            nc.sync.dma_start(out=idx_dst[:], in_=idx_dst_d[:, :])
            gsem = [nc.alloc_semaphore(f"swdge_dma{i}") for i in range(nqueues)]
            semcnt = [0] * nqueues

            # onehot resident (scatter lhsT), loaded once in 4 chunks
            oh_sb = const.tile([P, tot_tiles, P], bf16, tag="ohsb")
            ohr = oh_d.rearrange("(b p) n -> p b n", p=P)
            ck = math.ceil(tot_tiles / 4)
            for i in range(4):
                a, b = i * ck, min((i + 1) * ck, tot_tiles)
                if a < b:
                    nc.sync.dma_start(out=oh_sb[:, a:b, :], in_=ohr[:, a:b, :])

            h_t = nodes.tile([P, NT, D], fp32, tag="h")
            skip_t = nodes.tile([P, NT, D], bf16, tag="skip")
            hc_t = nodes.tile([P, NT, D], bf16, tag="hc")
            hnT_t = nodes.tile([P, NT, D], bf16, tag="hnT")

            def ln_stats(src_t):
                """Batched LN stats over all NT tiles of src_t.
                Returns (negmr, rstd): -mean*rstd and 1/std, both [P,NT]."""
                mv = stat.tile([P, NT, 2], fp32, tag="mv", name="mv")
                for t in range(NT):
                    st = nsm.tile([P, 6], fp32, tag="st", name="st")
                    nc.vector.bn_stats(out=st[:], in_=src_t[:, t, :])
                    nc.vector.bn_aggr(out=mv[:, t, :], in_=st[:])
                sd = stat.tile([P, NT], fp32, tag="sd", name="sd")
                nc.scalar.activation(out=sd[:], in_=mv[:, :, 1], func=AF.Sqrt,
                                     bias=eps_t[:], scale=1.0)
                rstd = stat.tile([P, NT], fp32, tag="rstd", name="rstd")
                nc.vector.reciprocal(out=rstd[:], in_=sd[:])
                negmr = stat.tile([P, NT], fp32, tag="negmr", name="negmr")
                nc.vector.scalar_tensor_tensor(
                    out=negmr[:], in0=mv[:, :, 0], scalar=-1.0,
                    in1=rstd[:], op0=OP.mult, op1=OP.mult)
                return negmr, rstd

            def ln_apply(src_ap, negmr, rstd, t, out_ap):
                # out = src*rstd - mean*rstd  (on ACT; Identity is in every
                # act table so this never triggers a table load)
                nc.scalar.activation(
                    out=out_ap, in_=src_ap, func=AF.Identity,
                    scale=rstd[:, t:t + 1], bias=negmr[:, t:t + 1])

            def elu_from_psum(ps_ap, out_ap):
                mn = nsm.tile([P, D], fp32, tag="mn", name="mn")
                nc.vector.tensor_scalar_min(mn[:], ps_ap, 0.0)
                em = nsm.tile([P, D], fp32, tag="em", name="em")
                nc.scalar.activation(out=em[:], in_=mn[:], func=AF.Exp)
                mx = nsm.tile([P, D], fp32, tag="mx", name="mx")
                nc.vector.tensor_scalar_max(mx[:], ps_ap, 0.0)
                nc.vector.scalar_tensor_tensor(
                    out=out_ap, in0=em[:], scalar=-1.0, in1=mx[:],
                    op0=OP.add, op1=OP.add)

            # ---------------- input projection (fp32) ----------------
            w0 = const.tile([F, D], fp32, tag="w0")
            nc.sync.dma_start(out=w0[:], in_=w0_d[:, :])
            with tc.tile_pool(name="ps0", bufs=2, space="PSUM") as ps0:
                for t in range(NT):
                    xt = ntmp.tile([P, F], fp32, tag="xt", name="xt")
                    nc.sync.dma_start(out=xt[:],
                                      in_=x_in[t * P:(t + 1) * P, :])
                    tp = ps0.tile([P, P], fp32, space="PSUM", tag="tp",
                                  name="tp")
                    nc.tensor.transpose(out=tp[:F, :], in_=xt[:],
                                        identity=id32[:])
                    xT = ntmp.tile([F, P], fp32, tag="xT", name="xT")
                    nc.scalar.copy(out=xT[:], in_=tp[:F, :])
                    h0 = ps0.tile([P, D], fp32, space="PSUM", tag="mm",
                                  name="h0")
                    nc.tensor.matmul(out=h0[:], lhsT=xT[:], rhs=w0[:],
                                     start=True, stop=True)
                    elu_from_psum(h0[:], h_t[:, t, :])

            def dump(name, src_t, cols):
                if not debug:
                    return
                for t in range(NT):
                    nc.sync.dma_start(out=dbg[name][t * P:(t + 1) * P, :],
                                      in_=src_t[:, t, :cols])

            dump("dbg_h0", h_t, D)

            # ---------------- layers ----------------
            for l in range(L):
                wkv = wpool.tile([D, 2 * D], bf16, tag="wkv", name="wkv")
                nc.sync.dma_start(out=wkv[:], in_=wkv_d[l])
                wqs = wpool.tile([D, 2 * D], bf16, tag="wqs", name="wqs")
                nc.sync.dma_start(out=wqs[:], in_=wqs_d[l])
                w1 = wpool.tile([D, D], bf16, tag="w1", name="w1")
                nc.sync.dma_start(out=w1[:], in_=w1_d[l])
                w2 = wpool.tile([D, D], bf16, tag="w2", name="w2")
                nc.sync.dma_start(out=w2[:], in_=w2_d[l])
                ewd = wpool.tile([ED, 2 * D], bf16, tag="ewd", name="ewd")
                nc.sync.dma_start(out=ewd[:], in_=ewd_d[l])

                # ---- node phase: LN1 + kv sweep ----
                mv1, rstd1 = ln_stats(h_t)
                with tc.tile_pool(name=f"npsA{l}", bufs=2, space="PSUM") \
                        as nps:
                    for t in range(NT):
                        hn = ntmp.tile([P, D], bf16, tag="hn", name="hn")
                        ln_apply(h_t[:, t, :], mv1, rstd1, t, hn[:])
                        tp = nps.tile([P, P], bf16, space="PSUM", tag="tp",
                                      name="tp")
                        nc.tensor.transpose(out=tp[:], in_=hn[:],
                                            identity=id16[:])
                        nc.scalar.copy(out=hnT_t[:, t, :], in_=tp[:])
                        kvp = nps.tile([P, 2 * D], fp32, space="PSUM",
                                       tag="mm", name="kvp")
                        nc.tensor.matmul(out=kvp[:], lhsT=hnT_t[:, t, :],
                                         rhs=wkv[:], start=True, stop=True)
                        kvb = ntmp.tile([P, 2 * D], bf16, tag="kvb",
                                        name="kvb")
                        nc.scalar.copy(out=kvb[:], in_=kvp[:])
                        nc.sync.dma_start(out=kv_bounce[t * P:(t + 1) * P, :],
                                          in_=kvb[:])
                        if debug and l == 0:
                            nc.sync.dma_start(
                                out=dbg["dbg_kv"][t * P:(t + 1) * P, :],
                                in_=kvb[:])

                # ---- kv exchange ----
                nc.gpsimd.collective_compute(
                    "AllGather", OP.bypass,
                    replica_groups=[list(range(NCORES))],
                    ins=[kv_bounce.opt()], outs=[kv_full.opt()])

                # ---- q|skip sweep (overlaps AllGather) ----
                with tc.tile_pool(name=f"npsB{l}", bufs=2, space="PSUM") \
                        as nps2:
                    for t in range(NT):
                        qsp = nps2.tile([P, 2 * D], fp32, space="PSUM",
                                        tag="mm", name="qsp")
                        nc.tensor.matmul(out=qsp[:], lhsT=hnT_t[:, t, :],
                                         rhs=wqs[:], start=True, stop=True)
                        qb = ntmp.tile([P, D], bf16, tag="qb", name="qb")
                        nc.scalar.copy(out=qb[:], in_=qsp[:, :D])
                        nc.sync.dma_start(out=q_tab[t * P:(t + 1) * P, :],
                                          in_=qb[:])
                        nc.vector.tensor_copy(out=skip_t[:, t, :],
                                              in_=qsp[:, D:])
                        if debug and l == 0:
                            nc.sync.dma_start(
                                out=dbg["dbg_q"][t * P:(t + 1) * P, :],
                                in_=qb[:])
                if debug and l == 0:
                    dump("dbg_skip", skip_t, D)

                # ---- edge phase ----
                # gpsimd executes in program order; these dummy reads carry
                # the RAW deps on the AllGather output / q_tab writes so the
                # deferred-trigger gather DMAs below cannot fire early.
                tok = nsm.tile([1, 2 * D], bf16, tag="tok", name="tok")
                nc.gpsimd.dma_start(out=tok[:], in_=kv_full[0:1, :])
                tok2 = nsm.tile([1, D], bf16, tag="tok2", name="tok2")
                nc.gpsimd.dma_start(out=tok2[:], in_=q_tab[0:1, :])
                with tc.tile_pool(name=f"epsK{l}", bufs=2, space="PSUM") \
                        as eps_ps, \
                        tc.tile_pool(name=f"epsA{l}", bufs=2, space="PSUM") \
                        as acc_ps:
                    acc_tiles = {}
                    for g in range(nbatch):
                        t0 = g * G
                        gb = min(G, tot_tiles - t0)
                        ne = gb * 128
                        qa = (2 * g) % nqueues
                        qb_ = (2 * g + 1) % nqueues
                        kvg = gbuf.tile([P, G, 2 * D], bf16, tag="kvg",
                                        name="kvg")
                        nc.gpsimd.dma_gather(
                            kvg[:, :gb, :], kv_full[:],
                            idx_src[:, t0 * 8:t0 * 8 + ne // 16],
                            ne, ne, 2 * D, queue_num=qa,
                            prepare_only=True, sem=gsem[qa])
                        nc.gpsimd.trigger_dma(count=None, queue_num=qa)
                        semcnt[qa] += 16
                        kv_tgt = semcnt[qa]
                        qgt = gbuf.tile([P, G, D], bf16, tag="qgt",
                                        name="qgt")
                        nc.gpsimd.dma_gather(
                            qgt[:, :gb, :], q_tab[:],
                            idx_dst[:, t0 * 8:t0 * 8 + ne // 16],
                            ne, ne, D, queue_num=qb_,
                            prepare_only=True, sem=gsem[qb_])
                        nc.gpsimd.trigger_dma(count=None, queue_num=qb_)
                        semcnt[qb_] += 16
                        q_tgt = semcnt[qb_]
                        eat = gbuf.tile([ED, G * 128], bf16, tag="eat",
                                        name="eat")
                        nc.sync.dma_start(
                            out=eat[:, :ne],
                            in_=ea_d[:, t0 * 128:t0 * 128 + ne])

                        for bb in range(math.ceil(gb / B)):
                            nb = min(B, gb - bb * B)
                            kvpe = eps_ps.tile([P, B, 2 * D], fp32,
                                               space="PSUM", tag="kvpe",
                                               name="kvpe")
                            if bb == 0:
                                nc.tensor.wait_ge(gsem[qa], kv_tgt)
                            # NOTE: each kvpe[:, u, :] start..stop pair must
                            # stay contiguous on the PE: a start=True matmul
                            # appears to clear has_written at bank granularity,
                            # so interleaving another start into the same bank
                            # corrupts an open accumulation group.
                            for u in range(nb):
                                te = bb * B + u
                                nc.tensor.matmul(
                                    out=kvpe[:, u, :],
                                    lhsT=eat[:, te * 128:(te + 1) * 128],
                                    rhs=ewd[:], start=True, stop=False,
                                    skip_group_check=True)
                                nc.tensor.matmul(
                                    out=kvpe[:, u, :], lhsT=id16[:],
                                    rhs=kvg[:, te, :], start=False, stop=True,
                                    skip_group_check=True)
                            if bb == 0:
                                nc.vector.wait_ge(gsem[qb_], q_tgt)
                            qk = ebuf.tile([P, B, D], bf16, tag="qk",
                                           name="qk")
                            nc.vector.tensor_tensor(
                                out=qk[:, :nb, :].rearrange(
                                    "p b (h c) -> p b h c", h=H),
                                in0=qgt[:, bb * B:bb * B + nb, :].rearrange(
                                    "p b (h c) -> p b h c", h=H),
                                in1=kvpe[:, :nb, :D].rearrange(
                                    "p b (h c) -> p b h c", h=H),
                                op=OP.mult)
                            al = ebuf.tile([P, B, H], bf16, tag="al",
                                           name="al")
                            with nc.allow_low_precision(
                                    reason="DVE reduces fp32 internally; "
                                    "bf16 is output rounding only"):
                                nc.vector.tensor_reduce(
                                    out=al[:, :nb, :],
                                    in_=qk[:, :nb, :].rearrange(
                                        "p b (h c) -> p b h c", h=H),
                                    axis=AX.X, op=OP.add)
                            pk = ebuf.tile([P, B, D + 8], bf16, tag="pk",
                                           name="pk")
                            nc.scalar.activation(
                                out=pk[:, :nb, D:], in_=al[:, :nb, :],
                                func=AF.Exp, scale=1.0 / math.sqrt(C))
                            nc.vector.tensor_tensor(
                                out=pk[:, :nb, :D].rearrange(
                                    "p b (h c) -> p b h c", h=H),
                                in0=kvpe[:, :nb, D:].rearrange(
                                    "p b (h c) -> p b h c", h=H),
                                in1=_bcast4(pk[:, :nb, D:], C),
                                op=OP.mult)
                            if debug and l == 0:
                                for u in range(nb):
                                    tid = t0 + bb * B + u
                                    nc.sync.dma_start(
                                        out=dbg["dbg_qg"][
                                            tid * 128:(tid + 1) * 128, :],
                                        in_=qgt[:, bb * B + u, :])
                                    nc.sync.dma_start(
                                        out=dbg["dbg_pk"][
                                            tid * 128:(tid + 1) * 128, :],
                                        in_=pk[:, u, :])
                            for u in range(nb):
                                tid = t0 + bb * B + u
                                w = tile_win[tid]
                                if win_first[tid]:
                                    acc_tiles[w] = acc_ps.tile(
                                        [P, D + 8], fp32, space="PSUM",
                                        tag="acc", name="acc")
                                nc.tensor.matmul(
                                    out=acc_tiles[w][:],
                                    lhsT=oh_sb[:, tid, :],
                                    rhs=pk[:, u, :],
                                    start=win_first[tid], stop=win_last[tid],
                                    skip_group_check=True)
                                if win_last[tid]:
                                    ac = acc_tiles.pop(w)
                                    dn = nsm.tile([P, H], fp32, tag="dn",
                                                  name="dn")
                                    nc.scalar.activation(
                                        out=dn[:], in_=ac[:, D:],
                                        func=AF.Copy, bias=1e-16)
                                    rd = nsm.tile([P, H], fp32, tag="rd",
                                                  name="rd")
                                    nc.vector.reciprocal(out=rd[:], in_=dn[:])
                                    mg = ntmp.tile([P, D], fp32, tag="mg",
                                                   name="mg")
                                    nc.vector.tensor_tensor(
                                        out=mg[:].rearrange(
                                            "p (h c) -> p h c", h=H),
                                        in0=ac[:, :D].rearrange(
                                            "p (h c) -> p h c", h=H),
                                        in1=_bcast3(rd[:], C), op=OP.mult)
                                    nc.vector.tensor_tensor(
                                        out=hc_t[:, w, :], in0=mg[:],
                                        in1=skip_t[:, w, :], op=OP.add)

                if debug and l == 0:
                    dump("dbg_hc", hc_t, D)

                # ---- FFN sweep 1: gelu, h += gelu(hc @ w1) ----
                with tc.tile_pool(name=f"npsC{l}", bufs=2, space="PSUM") \
                        as fps:
                    for t in range(NT):
                        tp = fps.tile([P, P], bf16, space="PSUM", tag="tp",
                                      name="tp")
                        nc.tensor.transpose(out=tp[:], in_=hc_t[:, t, :],
                                            identity=id16[:])
                        hcT = ntmp.tile([P, P], bf16, tag="hcT", name="hcT")
                        nc.scalar.copy(out=hcT[:], in_=tp[:])
                        t1p = fps.tile([P, D], fp32, space="PSUM", tag="mm",
                                       name="t1p")
                        nc.tensor.matmul(out=t1p[:], lhsT=hcT[:], rhs=w1[:],
                                         start=True, stop=True)
                        t1g = ntmp.tile([P, D], fp32, tag="t1g", name="t1g")
                        nc.scalar.activation(out=t1g[:], in_=t1p[:],
                                             func=AF.Gelu)
                        nc.vector.tensor_tensor(out=h_t[:, t, :], in0=t1g[:],
                                                in1=h_t[:, t, :], op=OP.add)

                if debug and l == 0:
                    dump("dbg_h1", h_t, D)

                # ---- FFN sweep 2: LN2 (batched) + elu residual ----
                mv2, rstd2 = ln_stats(h_t)
                with tc.tile_pool(name=f"npsD{l}", bufs=2, space="PSUM") \
                        as fps2:
                    for t in range(NT):
                        t2 = ntmp.tile([P, D], bf16, tag="t2", name="t2")
                        ln_apply(h_t[:, t, :], mv2, rstd2, t, t2[:])
                        tp = fps2.tile([P, P], bf16, space="PSUM", tag="tp",
                                       name="tp")
                        nc.tensor.transpose(out=tp[:], in_=t2[:],
                                            identity=id16[:])
                        t2T = ntmp.tile([P, P], bf16, tag="t2T", name="t2T")
                        nc.scalar.copy(out=t2T[:], in_=tp[:])
                        t3p = fps2.tile([P, D], fp32, space="PSUM", tag="mm",
                                        name="t3p")
                        nc.tensor.matmul(out=t3p[:], lhsT=t2T[:], rhs=w2[:],
                                         start=True, stop=True)
                        t4 = ntmp.tile([P, D], fp32, tag="t4", name="t4")
                        elu_from_psum(t3p[:], t4[:])
                        nc.vector.tensor_tensor(out=h_t[:, t, :], in0=t4[:],
                                                in1=h_t[:, t, :], op=OP.add)
                if debug and l == 0:
                    dump("dbg_h2", h_t, D)

            # ---------------- output head ----------------
            wl = const.tile([D, 4], bf16, tag="wl")
            nc.sync.dma_start(out=wl[:], in_=wl_d[:, :])
            mvh, rstdh = ln_stats(h_t)
            with tc.tile_pool(name="psH", bufs=2, space="PSUM") as psh:
                for t in range(NT):
                    hn = ntmp.tile([P, D], bf16, tag="hn", name="hnl")
                    ln_apply(h_t[:, t, :], mvh, rstdh, t, hn[:])
                    tp = psh.tile([P, P], bf16, space="PSUM", tag="tp",
                                  name="tp")
                    nc.tensor.transpose(out=tp[:], in_=hn[:],
                                        identity=id16[:])
                    hnT = ntmp.tile([P, P], bf16, tag="hnT2", name="hnT2")
                    nc.scalar.copy(out=hnT[:], in_=tp[:])
                    op_ = psh.tile([P, 4], fp32, space="PSUM", tag="mm",
                                   name="op")
                    nc.tensor.matmul(out=op_[:], lhsT=hnT[:], rhs=wl[:],
                                     start=True, stop=True)
                    ot = ntmp.tile([P, 4], fp32, tag="ot", name="ot")
                    nc.scalar.copy(out=ot[:], in_=op_[:])
                    nc.sync.dma_start(out=out_d[t * P:(t + 1) * P, :],
                                      in_=ot[:])

    nc.compile()
    return nc


def prep_inputs(x, edge_index, edge_attr,
                lin0_w, lin0_b,
                q_w, q_b, k_w, k_b, v_w, v_b, e_w, skip_w, skip_b,
                ln1_g, ln1_b, lins_w, lins_b, ln2_g, ln2_b,
                lins2_w, lins2_b, lnl_g, lnl_b, linl_w, linl_b):
    """Host-side sharding/sorting/folding."""
    x = np.asarray(x, np.float32)
    ei = np.asarray(edge_index, np.int64)
    ea = np.asarray(edge_attr, np.float32)
    src, dst = ei[0], ei[1]
    core = dst // NL
    slot = dst - core * NL

    def fold(W, bias, g, b):
        W = np.asarray(W, np.float64)
        Wf = W * np.asarray(g, np.float64)[None, :]
        cf = np.asarray(bias, np.float64) + W @ np.asarray(b, np.float64)
        return Wf.astype(np.float32), cf.astype(np.float32)

    wkvT = np.zeros((L, D, 2 * D), np.float32)
    wqsT = np.zeros((L, D, 2 * D), np.float32)
    w1T = np.zeros((L, D, D), np.float32)
    w2T = np.zeros((L, D, D), np.float32)
    ewdT = np.zeros((L, ED, 2 * D), np.float32)
    zero_bias = True
    for l in range(L):
        for (W, bias, dstT, half) in [
                (k_w[l], k_b[l], wkvT, 0), (v_w[l], v_b[l], wkvT, 1),
                (q_w[l], q_b[l], wqsT, 0), (skip_w[l], skip_b[l], wqsT, 1)]:
            Wf, cf = fold(W, bias, ln1_g[l], ln1_b[l])
            dstT[l, :, half * D:(half + 1) * D] = Wf.T
            zero_bias &= bool(np.abs(cf).max() == 0)
        w1T[l] = np.asarray(lins_w[l]).T
        zero_bias &= bool(np.abs(np.asarray(lins_b[l])).max() == 0)
        Wf, cf = fold(lins2_w[l], lins2_b[l], ln2_g[l], ln2_b[l])
        w2T[l] = Wf.T
        zero_bias &= bool(np.abs(cf).max() == 0)
        ewT = np.asarray(e_w[l]).T.astype(np.float32)   # [ED, D]
        ewdT[l, :, :D] = ewT
        ewdT[l, :, D:] = ewT
    Wl, cl = fold(linl_w, linl_b, lnl_g, lnl_b)
    wlT = np.zeros((D, 4), np.float32)
    wlT[:, :3] = Wl.T
    zero_bias &= bool(np.abs(cl).max() == 0)
    zero_bias &= bool(np.abs(np.asarray(lin0_b)).max() == 0)

    win = slot // 128
    counts = np.zeros((NCORES, NT), np.int64)
    np.add.at(counts, (core, win), 1)
    tiles_per_window = [max(1, int(math.ceil(counts[:, w].max() / 128)))
                        for w in range(NT)]
    tot_tiles = sum(tiles_per_window)
    tot_e = tot_tiles * 128

    in_maps = []
    order_all = np.lexsort((win, core))
    off = np.searchsorted(core[order_all], np.arange(NCORES + 1))
    kvrow_of = (src // NL) * NLP + (src % NL)

    for c in range(NCORES):
        oc = order_all[off[c]:off[c + 1]]
        wc = win[oc]
        woff = np.searchsorted(wc, np.arange(NT + 1))
        src_rows = np.zeros(tot_e, np.int16)
        dst_rows = np.zeros(tot_e, np.int16)
        onehot = np.zeros((tot_e, P), np.float32)
        ea_t = np.zeros((ED, tot_e), np.float32)
        base = 0
        for w in range(NT):
            ew_idx = oc[woff[w]:woff[w + 1]]
            k = len(ew_idx)
            sl = slice(base, base + k)
            src_rows[sl] = kvrow_of[ew_idx].astype(np.int16)
            dst_rows[sl] = slot[ew_idx].astype(np.int16)
            onehot[np.arange(base, base + k), slot[ew_idx] - w * 128] = 1.0
            ea_t[:, sl] = ea[ew_idx].T
            base += tiles_per_window[w] * 128
        assert base == tot_e

        def wrap(a):
            return np.tile(a.reshape(tot_e // 16, 16).T, (8, 1)).copy()

        xs = np.zeros((NLP, F), np.float32)
        xs[:NL] = x[c * NL:(c + 1) * NL]
        in_maps.append({
            "x_shard": xs,
            "idx_src": wrap(src_rows),
            "idx_dst": wrap(dst_rows),
            "onehot": onehot,
            "ea_t": ea_t,
            "wkvT": wkvT, "wqsT": wqsT,
            "w1T": w1T, "w2T": w2T, "ewdT": ewdT,
            "w0T": np.asarray(lin0_w).T.astype(np.float32),
            "ident": np.eye(P, dtype=np.float32),
            "wlT": wlT,
        })
    return in_maps, tiles_per_window, zero_bias


_CACHE = {}
LAST_RES = None


def kernel(**inputs):
    global LAST_RES
    import ml_dtypes
    in_maps, tiles_per_window, zero_bias = prep_inputs(**inputs)
    for m in in_maps:
        for k in ("onehot", "ea_t", "wkvT", "wqsT", "w1T", "w2T", "ewdT",
                  "wlT"):
            m[k] = m[k].astype(ml_dtypes.bfloat16)

    import os
    debug = bool(int(os.environ.get("K_DEBUG", "0")))
    key = (tuple(tiles_per_window), debug)
    if key not in _CACHE:
        _CACHE[key] = build(tiles_per_window, zero_bias, debug)
    nc = _CACHE[key]

    res = run_bass_kernel_spmd(nc, in_maps, core_ids=list(range(NCORES)))
    LAST_RES = res
    out = np.zeros((N, 3), np.float32)
    for c in range(NCORES):
        out[c * NL:(c + 1) * NL] = res.results[c]["out"][:NL, :3]
    return out


# revision 55
# speedup vs baseline: 1.9949x; 1.9949x over previous
"""TransformerConv GNN (3 layers) on 8 Trainium2 NeuronCores.

Sharding: nodes split 3750/core (padded to 3840 = 30 tiles of 128).
Edges assigned to the core owning their dst node, grouped by 128-node
dst windows. Per layer:
  node phase: batched LN1 stats (one sqrt table load), kv projection
    sweep (bf16 matmuls, fused k|v weights) -> kv_bounce HBM; AllGather;
    q|skip projection sweep overlaps the AllGather.
  edge phase: dma_gather of kv[src] (4 SWDGE queues, 4-deep pipeline);
    onehot tiles resident in SBUF (scatter lhsT); transposed onehot
    loaded per batch via DMA-transpose (gather lhsT); q gathered on the
    PE via onehot matmul from SBUF-resident q; edge-attr projection on
    PE accumulated with gathered k|v in PSUM; attention alpha/exp/
    message on DVE+ACT; segment softmax accumulated per dst window via
    one-hot matmuls into PSUM.
  FFN phase: two sweeps (gelu sweep, then batched-LN2 + elu sweep) to
    avoid ACT table thrash; h updated in place.
Output head node-local; host reassembles shards.
"""
import contextlib
import math
import numpy as np

import concourse.bass as bass
import concourse.bacc as bacc
import concourse.tile as tile
from concourse import mybir, library_config
from concourse.bass_utils import run_bass_kernel_spmd

# problem dims
N, E, F, D, H, C, ED, L = 30000, 300000, 64, 128, 8, 16, 16, 3
NCORES = 8
NL = N // NCORES          # 3750 real nodes per core
NT = 30                   # node tiles per core
NLP = NT * 128            # 3840 padded nodes per core
KVROWS = NCORES * NLP     # kv table rows (global)
P = 128
G = 8                     # edge tiles per gather batch (dma_gather fails >1024 idxs/call)
B = 4                     # edge tiles per DVE op group

fp32 = mybir.dt.float32
bf16 = mybir.dt.bfloat16
i16 = mybir.dt.int16

AF = mybir.ActivationFunctionType
OP = mybir.AluOpType
AX = mybir.AxisListType


def _bcast3(ap, reps):
    """[P, k] AP -> [P, k, reps] with 0-stride last dim."""
    return bass.AP(tensor=ap.tensor, offset=ap.offset,
                   ap=[ap.ap[0], ap.ap[1], [0, reps]])


def _bcast4(ap, reps):
    """[P, b, k] AP -> [P, b, k, reps] with 0-stride last dim."""
    return bass.AP(tensor=ap.tensor, offset=ap.offset,
                   ap=[ap.ap[0], ap.ap[1], ap.ap[2], [0, reps]])


def build(tiles_per_window, skip_bias, debug=False):
    """Build the Bass program. tiles_per_window: NT ints, same per core."""
    import os
    nqueues = int(os.environ.get("K_Q", "4"))
    NPRE = int(os.environ.get("K_NPRE", "0"))
    assert skip_bias, "non-zero bias path not implemented"
    tot_tiles = sum(tiles_per_window)
    tot_e = tot_tiles * 128
    nbatch = math.ceil(tot_tiles / G)

    tile_win, win_first, win_last = [], [], []
    for w, tw in enumerate(tiles_per_window):
        for i in range(tw):
            tile_win.append(w)
            win_first.append(i == 0)
            win_last.append(i == tw - 1)

    nc = bacc.Bacc("TRN2", target_bir_lowering=False, debug=False,
                   num_devices=NCORES, num_swdge_queues=nqueues)

    # ---------------- DRAM tensors ----------------
    x_in = nc.dram_tensor("x_shard", [NLP, F], fp32, kind="ExternalInput").ap()
    idx_src_d = nc.dram_tensor("idx_src", [P, tot_e // 16], i16,
                               kind="ExternalInput").ap()
    oh_d = nc.dram_tensor("onehot", [tot_e, P], bf16, kind="ExternalInput").ap()
    ea_d = nc.dram_tensor("ea_t", [ED, tot_e], bf16, kind="ExternalInput").ap()
    wkv_d = nc.dram_tensor("wkvT", [L, D, 2 * D], bf16,
                           kind="ExternalInput").ap()
    wqs_d = nc.dram_tensor("wqsT", [L, D, 2 * D], bf16,
                           kind="ExternalInput").ap()
    w1_d = nc.dram_tensor("w1T", [L, D, D], bf16, kind="ExternalInput").ap()
    w2_d = nc.dram_tensor("w2T", [L, D, D], bf16, kind="ExternalInput").ap()
    ewd_d = nc.dram_tensor("ewdT", [L, ED, 2 * D], bf16,
                           kind="ExternalInput").ap()
    w0_d = nc.dram_tensor("w0T", [F, D], fp32, kind="ExternalInput").ap()
    id_d = nc.dram_tensor("ident", [P, P], fp32, kind="ExternalInput").ap()
    wl_d = nc.dram_tensor("wlT", [D, 4], bf16, kind="ExternalInput").ap()
    out_d = nc.dram_tensor("out", [NLP, 4], fp32, kind="ExternalOutput").ap()

    kv_bounce = nc.dram_tensor("kv_bounce", [NLP, 2 * D], bf16).ap()
    kv_full = nc.dram_tensor("kv_full", [KVROWS, 2 * D], bf16,
                             addr_space="Shared").ap()

    dbg = {}
    if debug:
        for nm, sh, dt in [("dbg_h0", [NLP, D], fp32),
                           ("dbg_kv", [NLP, 2 * D], bf16),
                           ("dbg_q", [NLP, D], bf16),
                           ("dbg_skip", [NLP, D], bf16),
                           ("dbg_hc", [NLP, D], bf16),
                           ("dbg_h1", [NLP, D], fp32),
                           ("dbg_h2", [NLP, D], fp32),
                           ("dbg_qg", [tot_e, D], bf16),
                           ("dbg_pk", [tot_e, D + 8], bf16)]:
            dbg[nm] = nc.dram_tensor(nm, sh, dt, kind="ExternalOutput").ap()

    eps = 1e-5

    with tile.TileContext(nc) as tc:
        nc.gpsimd.load_library(library_config.mlp)
        with contextlib.ExitStack() as ctx:
            const = ctx.enter_context(tc.tile_pool(name="const", bufs=1))
            nodes = ctx.enter_context(tc.tile_pool(name="nodes", bufs=1))
            wpool = ctx.enter_context(tc.tile_pool(name="wpool", bufs=2))
            ntmp = ctx.enter_context(tc.tile_pool(name="ntmp", bufs=3))
            nsm = ctx.enter_context(tc.tile_pool(name="nsm", bufs=4))
            stat = ctx.enter_context(tc.tile_pool(name="stat", bufs=2))
            gbuf = ctx.enter_context(tc.tile_pool(name="gbuf", bufs=3))
            kvpool = ctx.enter_context(
                tc.tile_pool(name="kvpool", bufs=max(NPRE, 4)))
            ebuf = ctx.enter_context(tc.tile_pool(name="ebuf", bufs=3))

            # constants
            id32 = const.tile([P, P], fp32, tag="id32")
            nc.sync.dma_start(out=id32[:], in_=id_d[:, :])
            id16 = const.tile([P, P], bf16, tag="id16")
            nc.vector.tensor_copy(out=id16[:], in_=id32[:])
            eps_t = const.tile([P, 1], fp32, tag="eps")
            nc.vector.memset(eps_t[:], eps)

            idx_src = const.tile([P, tot_e // 16], i16, tag="isrc")
            nc.sync.dma_start(out=idx_src[:], in_=idx_src_d[:, :])
            gsem = [nc.alloc_semaphore(f"swdge_dma{i}") for i in range(nqueues)]
            semcnt = [0] * nqueues

            # onehot resident (scatter lhsT), loaded once in 4 chunks
            oh_sb = const.tile([P, tot_tiles, P], bf16, tag="ohsb")
            ohr = oh_d.rearrange("(b p) n -> p b n", p=P)
            ck = math.ceil(tot_tiles / 4)
            for i in range(4):
                a, b = i * ck, min((i + 1) * ck, tot_tiles)
                if a < b:
                    nc.sync.dma_start(out=oh_sb[:, a:b, :], in_=ohr[:, a:b, :])

            h_t = nodes.tile([P, NT, D], fp32, tag="h")
            skip_t = nodes.tile([P, NT, D], bf16, tag="skip")
            hc_t = nodes.tile([P, NT, D], bf16, tag="hc")
            hnT_t = nodes.tile([P, NT, D], bf16, tag="hnT")
            q_sb = nodes.tile([P, NT, D], bf16, tag="qsb")

            def ln_stats(src_t):
                """Batched LN stats over all NT tiles of src_t.
                Returns (negmr, rstd): -mean*rstd and 1/std, both [P,NT]."""
                mv = stat.tile([P, NT, 2], fp32, tag="mv", name="mv")
                for t in range(NT):
                    st = nsm.tile([P, 6], fp32, tag="st", name="st")
                    nc.vector.bn_stats(out=st[:], in_=src_t[:, t, :])
                    nc.vector.bn_aggr(out=mv[:, t, :], in_=st[:])
                sd = stat.tile([P, NT], fp32, tag="sd", name="sd")
                nc.scalar.activation(out=sd[:], in_=mv[:, :, 1], func=AF.Sqrt,
                                     bias=eps_t[:], scale=1.0)
                rstd = stat.tile([P, NT], fp32, tag="rstd", name="rstd")
                nc.vector.reciprocal(out=rstd[:], in_=sd[:])
                negmr = stat.tile([P, NT], fp32, tag="negmr", name="negmr")
                nc.vector.scalar_tensor_tensor(
                    out=negmr[:], in0=mv[:, :, 0], scalar=-1.0,
                    in1=rstd[:], op0=OP.mult, op1=OP.mult)
                return negmr, rstd

            def ln_apply(src_ap, negmr, rstd, t, out_ap):
                # out = src*rstd - mean*rstd  (on ACT; Identity is in every
                # act table so this never triggers a table load)
                nc.scalar.activation(
                    out=out_ap, in_=src_ap, func=AF.Identity,
                    scale=rstd[:, t:t + 1], bias=negmr[:, t:t + 1])

            def elu_from_psum(ps_ap, out_ap):
                mn = nsm.tile([P, D], fp32, tag="mn", name="mn")
                nc.vector.tensor_scalar_min(mn[:], ps_ap, 0.0)
                em = nsm.tile([P, D], fp32, tag="em", name="em")
                nc.scalar.activation(out=em[:], in_=mn[:], func=AF.Exp)
                mx = nsm.tile([P, D], fp32, tag="mx", name="mx")
                nc.vector.tensor_scalar_max(mx[:], ps_ap, 0.0)
                nc.vector.scalar_tensor_tensor(
                    out=out_ap, in0=em[:], scalar=-1.0, in1=mx[:],
                    op0=OP.add, op1=OP.add)

            # ---------------- input projection (fp32) ----------------
            w0 = const.tile([F, D], fp32, tag="w0")
            nc.sync.dma_start(out=w0[:], in_=w0_d[:, :])
            with tc.tile_pool(name="ps0", bufs=2, space="PSUM") as ps0:
                for t in range(NT):
                    xt = ntmp.tile([P, F], fp32, tag="xt", name="xt")
                    nc.sync.dma_start(out=xt[:],
                                      in_=x_in[t * P:(t + 1) * P, :])
                    tp = ps0.tile([P, P], fp32, space="PSUM", tag="tp",
                                  name="tp")
                    nc.tensor.transpose(out=tp[:F, :], in_=xt[:],
                                        identity=id32[:])
                    xT = ntmp.tile([F, P], fp32, tag="xT", name="xT")
                    nc.scalar.copy(out=xT[:], in_=tp[:F, :])
                    h0 = ps0.tile([P, D], fp32, space="PSUM", tag="mm",
                                  name="h0")
                    nc.tensor.matmul(out=h0[:], lhsT=xT[:], rhs=w0[:],
                                     start=True, stop=True)
                    elu_from_psum(h0[:], h_t[:, t, :])

            def dump(name, src_t, cols):
                if not debug:
                    return
                for t in range(NT):
                    nc.sync.dma_start(out=dbg[name][t * P:(t + 1) * P, :],
                                      in_=src_t[:, t, :cols])

            dump("dbg_h0", h_t, D)

            # ---------------- layers ----------------
            for l in range(L):
                wkv = wpool.tile([D, 2 * D], bf16, tag="wkv", name="wkv")
                nc.sync.dma_start(out=wkv[:], in_=wkv_d[l])
                wqs = wpool.tile([D, 2 * D], bf16, tag="wqs", name="wqs")
                nc.sync.dma_start(out=wqs[:], in_=wqs_d[l])
                w1 = wpool.tile([D, D], bf16, tag="w1", name="w1")
                nc.sync.dma_start(out=w1[:], in_=w1_d[l])
                w2 = wpool.tile([D, D], bf16, tag="w2", name="w2")
                nc.sync.dma_start(out=w2[:], in_=w2_d[l])
                ewd = wpool.tile([ED, 2 * D], bf16, tag="ewd", name="ewd")
                nc.sync.dma_start(out=ewd[:], in_=ewd_d[l])

                # ---- node phase: LN1 + kv sweep ----
                mv1, rstd1 = ln_stats(h_t)
                with tc.tile_pool(name=f"npsA{l}", bufs=2, space="PSUM") \
                        as nps:
                    for t in range(NT):
                        hn = ntmp.tile([P, D], bf16, tag="hn", name="hn")
                        ln_apply(h_t[:, t, :], mv1, rstd1, t, hn[:])
                        tp = nps.tile([P, P], bf16, space="PSUM", tag="tp",
                                      name="tp")
                        nc.tensor.transpose(out=tp[:], in_=hn[:],
                                            identity=id16[:])
                        nc.scalar.copy(out=hnT_t[:, t, :], in_=tp[:])
                        kvp = nps.tile([P, 2 * D], fp32, space="PSUM",
                                       tag="mm", name="kvp")
                        nc.tensor.matmul(out=kvp[:], lhsT=hnT_t[:, t, :],
                                         rhs=wkv[:], start=True, stop=True)
                        kvb = ntmp.tile([P, 2 * D], bf16, tag="kvb",
                                        name="kvb")
                        nc.scalar.copy(out=kvb[:], in_=kvp[:])
                        nc.sync.dma_start(out=kv_bounce[t * P:(t + 1) * P, :],
                                          in_=kvb[:])
                        if debug and l == 0:
                            nc.sync.dma_start(
                                out=dbg["dbg_kv"][t * P:(t + 1) * P, :],
                                in_=kvb[:])

                # ---- prep gather descriptors for the first NPRE batches
                # (desc-gen is ~8.6us/batch of serialized gpsimd work; doing
                # it before/during the AllGather hides it; triggers fire after
                # the dummy ordering read below) ----
                pre_kvg, pre_tgt = {}, {}
                for g in range(min(NPRE, nbatch)):
                    t0 = g * G
                    gb = min(G, tot_tiles - t0)
                    ne = gb * 128
                    qa = g % nqueues
                    kvg = kvpool.tile([P, G, 2 * D], bf16, tag="kvg",
                                      name="kvg")
                    nc.gpsimd.dma_gather(
                        kvg[:, :gb, :], kv_full[:],
                        idx_src[:, t0 * 8:t0 * 8 + ne // 16],
                        ne, ne, 2 * D, queue_num=qa,
                        prepare_only=True, sem=gsem[qa])
                    semcnt[qa] += 16
                    pre_kvg[g] = kvg
                    pre_tgt[g] = (qa, semcnt[qa])

                # ---- kv exchange ----
                nc.gpsimd.collective_compute(
                    "AllGather", OP.bypass,
                    replica_groups=[list(range(NCORES))],
                    ins=[kv_bounce.opt()], outs=[kv_full.opt()])

                # ---- q|skip sweep (overlaps AllGather) ----
                with tc.tile_pool(name=f"npsB{l}", bufs=2, space="PSUM") \
                        as nps2:
                    for t in range(NT):
                        qsp = nps2.tile([P, 2 * D], fp32, space="PSUM",
                                        tag="mm", name="qsp")
                        nc.tensor.matmul(out=qsp[:], lhsT=hnT_t[:, t, :],
                                         rhs=wqs[:], start=True, stop=True)
                        nc.scalar.copy(out=q_sb[:, t, :], in_=qsp[:, :D])
                        nc.vector.tensor_copy(out=skip_t[:, t, :],
                                              in_=qsp[:, D:])
                if debug and l == 0:
                    dump("dbg_q", q_sb, D)
                    dump("dbg_skip", skip_t, D)

                # ---- edge phase ----
                # gpsimd executes in program order; this dummy read carries
                # the RAW dep on the AllGather output so the deferred-trigger
                # gather DMAs below cannot fire early.
                tok = nsm.tile([1, 2 * D], bf16, tag="tok", name="tok")
                nc.gpsimd.dma_start(out=tok[:], in_=kv_full[0:1, :])
                for qn in range(nqueues):
                    if any(pre_tgt[g][0] == qn for g in pre_tgt):
                        nc.gpsimd.trigger_dma(count=None, queue_num=qn)
                with tc.tile_pool(name=f"epsK{l}", bufs=2, space="PSUM") \
                        as eps_ps, \
                        tc.tile_pool(name=f"epsA{l}", bufs=2, space="PSUM") \
                        as acc_ps, \
                        tc.tile_pool(name=f"epsQ{l}", bufs=2, space="PSUM") \
                        as qg_ps:
                    acc_tiles = {}
                    for g in range(nbatch):
                        t0 = g * G
                        gb = min(G, tot_tiles - t0)
                        ne = gb * 128
                        if g in pre_kvg:
                            kvg = pre_kvg[g]
                            kv_wait = pre_tgt[g]
                        else:
                            kvg = kvpool.tile([P, G, 2 * D], bf16, tag="kvg",
                                              name="kvg")
                            nc.gpsimd.dma_gather(
                                kvg[:, :gb, :], kv_full[:],
                                idx_src[:, t0 * 8:t0 * 8 + ne // 16],
                                ne, ne, 2 * D, queue_num=g % nqueues)
                            kv_wait = None
                        ohT_g = gbuf.tile([P, G, P], bf16, tag="ohT",
                                          name="ohT")
                        nc.sync.dma_start(
                            out=ohT_g[:, :gb, :],
                            in_=oh_d[t0 * 128:t0 * 128 + ne, :],
                            transpose=True)
                        eat = gbuf.tile([ED, G * 128], bf16, tag="eat",
                                        name="eat")
                        nc.sync.dma_start(
                            out=eat[:, :ne],
                            in_=ea_d[:, t0 * 128:t0 * 128 + ne])

                        for bb in range(math.ceil(gb / B)):
                            nb = min(B, gb - bb * B)
                            kvpe = eps_ps.tile([P, B, 2 * D], fp32,
                                               space="PSUM", tag="kvpe",
                                               name="kvpe")
                            if bb == 0 and kv_wait is not None:
                                nc.tensor.wait_ge(gsem[kv_wait[0]],
                                                  kv_wait[1])
                            # NOTE: each kvpe[:, u, :] start..stop pair must
                            # stay contiguous on the PE: a start=True matmul
                            # appears to clear has_written at bank granularity,
                            # so interleaving another start into the same bank
                            # corrupts an open accumulation group.
                            for u in range(nb):
                                te = bb * B + u
                                nc.tensor.matmul(
                                    out=kvpe[:, u, :],
                                    lhsT=eat[:, te * 128:(te + 1) * 128],
                                    rhs=ewd[:], start=True, stop=False,
                                    skip_group_check=True)
                                nc.tensor.matmul(
                                    out=kvpe[:, u, :], lhsT=id16[:],
                                    rhs=kvg[:, te, :], start=False, stop=True,
                                    skip_group_check=True)
                            # q gather via onehot matmul (complete start/stop
                            # groups in their own PSUM pool — safe to follow
                            # the closed kvpe groups)
                            qgb = ebuf.tile([P, B, D], bf16, tag="qgb",
                                            name="qgb")
                            for u in range(nb):
                                te = bb * B + u
                                w = tile_win[t0 + te]
                                qgp = qg_ps.tile([P, D], fp32, space="PSUM",
                                                 tag="qg", name="qgp")
                                nc.tensor.matmul(
                                    out=qgp[:], lhsT=ohT_g[:, te, :],
                                    rhs=q_sb[:, w, :], start=True, stop=True)
                                nc.scalar.copy(out=qgb[:, u, :], in_=qgp[:])
                            qk = ebuf.tile([P, B, D], bf16, tag="qk",
                                           name="qk")
                            nc.vector.tensor_tensor(
                                out=qk[:, :nb, :].rearrange(
                                    "p b (h c) -> p b h c", h=H),
                                in0=qgb[:, :nb, :].rearrange(
                                    "p b (h c) -> p b h c", h=H),
                                in1=kvpe[:, :nb, :D].rearrange(
                                    "p b (h c) -> p b h c", h=H),
                                op=OP.mult)
                            al = ebuf.tile([P, B, H], bf16, tag="al",
                                           name="al")
                            with nc.allow_low_precision(
                                    reason="DVE reduces fp32 internally; "
                                    "bf16 is output rounding only"):
                                nc.vector.tensor_reduce(
                                    out=al[:, :nb, :],
                                    in_=qk[:, :nb, :].rearrange(
                                        "p b (h c) -> p b h c", h=H),
                                    axis=AX.X, op=OP.add)
                            pk = ebuf.tile([P, B, D + 8], bf16, tag="pk",
                                           name="pk")
                            nc.scalar.activation(
                                out=pk[:, :nb, D:], in_=al[:, :nb, :],
                                func=AF.Exp, scale=1.0 / math.sqrt(C))
                            nc.vector.tensor_tensor(
                                out=pk[:, :nb, :D].rearrange(
                                    "p b (h c) -> p b h c", h=H),
                                in0=kvpe[:, :nb, D:].rearrange(
                                    "p b (h c) -> p b h c", h=H),
                                in1=_bcast4(pk[:, :nb, D:], C),
                                op=OP.mult)
                            if debug and l == 0:
                                for u in range(nb):
                                    tid = t0 + bb * B + u
                                    nc.sync.dma_start(
                                        out=dbg["dbg_qg"][
                                            tid * 128:(tid + 1) * 128, :],
                                        in_=qgb[:, u, :])
                                    nc.sync.dma_start(
                                        out=dbg["dbg_pk"][
                                            tid * 128:(tid + 1) * 128, :],
                                        in_=pk[:, u, :])
                            for u in range(nb):
                                tid = t0 + bb * B + u
                                w = tile_win[tid]
                                if win_first[tid]:
                                    acc_tiles[w] = acc_ps.tile(
                                        [P, D + 8], fp32, space="PSUM",
                                        tag="acc", name="acc")
                                nc.tensor.matmul(
                                    out=acc_tiles[w][:],
                                    lhsT=oh_sb[:, tid, :],
                                    rhs=pk[:, u, :],
                                    start=win_first[tid], stop=win_last[tid],
                                    skip_group_check=True)
                                if win_last[tid]:
                                    ac = acc_tiles.pop(w)
                                    dn = nsm.tile([P, H], fp32, tag="dn",
                                                  name="dn")
                                    nc.scalar.activation(
                                        out=dn[:], in_=ac[:, D:],
                                        func=AF.Copy, bias=1e-16)
                                    rd = nsm.tile([P, H], fp32, tag="rd",
                                                  name="rd")
                                    nc.vector.reciprocal(out=rd[:], in_=dn[:])
                                    mg = ntmp.tile([P, D], fp32, tag="mg",
                                                   name="mg")
                                    nc.vector.tensor_tensor(
                                        out=mg[:].rearrange(
                                            "p (h c) -> p h c", h=H),
                                        in0=ac[:, :D].rearrange(
                                            "p (h c) -> p h c", h=H),
                                        in1=_bcast3(rd[:], C), op=OP.mult)
                                    nc.vector.tensor_tensor(
                                        out=hc_t[:, w, :], in0=mg[:],
                                        in1=skip_t[:, w, :], op=OP.add)

                if debug and l == 0:
                    dump("dbg_hc", hc_t, D)

                # ---- FFN sweep 1: gelu, h += gelu(hc @ w1) ----
                with tc.tile_pool(name=f"npsC{l}", bufs=2, space="PSUM") \
                        as fps:
                    for t in range(NT):
                        tp = fps.tile([P, P], bf16, space="PSUM", tag="tp",
                                      name="tp")
                        nc.tensor.transpose(out=tp[:], in_=hc_t[:, t, :],
                                            identity=id16[:])
                        hcT = ntmp.tile([P, P], bf16, tag="hcT", name="hcT")
                        nc.scalar.copy(out=hcT[:], in_=tp[:])
                        t1p = fps.tile([P, D], fp32, space="PSUM", tag="mm",
                                       name="t1p")
                        nc.tensor.matmul(out=t1p[:], lhsT=hcT[:], rhs=w1[:],
                                         start=True, stop=True)
                        t1g = ntmp.tile([P, D], fp32, tag="t1g", name="t1g")
                        nc.scalar.activation(out=t1g[:], in_=t1p[:],
                                             func=AF.Gelu)
                        nc.vector.tensor_tensor(out=h_t[:, t, :], in0=t1g[:],
                                                in1=h_t[:, t, :], op=OP.add)

                if debug and l == 0:
                    dump("dbg_h1", h_t, D)

                # ---- FFN sweep 2: LN2 (batched) + elu residual ----
                mv2, rstd2 = ln_stats(h_t)
                with tc.tile_pool(name=f"npsD{l}", bufs=2, space="PSUM") \
                        as fps2:
                    for t in range(NT):
                        t2 = ntmp.tile([P, D], bf16, tag="t2", name="t2")
                        ln_apply(h_t[:, t, :], mv2, rstd2, t, t2[:])
                        tp = fps2.tile([P, P], bf16, space="PSUM", tag="tp",
                                       name="tp")
                        nc.tensor.transpose(out=tp[:], in_=t2[:],
                                            identity=id16[:])
                        t2T = ntmp.tile([P, P], bf16, tag="t2T", name="t2T")
                        nc.scalar.copy(out=t2T[:], in_=tp[:])
                        t3p = fps2.tile([P, D], fp32, space="PSUM", tag="mm",
                                        name="t3p")
                        nc.tensor.matmul(out=t3p[:], lhsT=t2T[:], rhs=w2[:],
                                         start=True, stop=True)
                        t4 = ntmp.tile([P, D], fp32, tag="t4", name="t4")
                        elu_from_psum(t3p[:], t4[:])
                        nc.vector.tensor_tensor(out=h_t[:, t, :], in0=t4[:],
                                                in1=h_t[:, t, :], op=OP.add)
                if debug and l == 0:
                    dump("dbg_h2", h_t, D)

            # ---------------- output head ----------------
            wl = const.tile([D, 4], bf16, tag="wl")
            nc.sync.dma_start(out=wl[:], in_=wl_d[:, :])
            mvh, rstdh = ln_stats(h_t)
            with tc.tile_pool(name="psH", bufs=2, space="PSUM") as psh:
                for t in range(NT):
                    hn = ntmp.tile([P, D], bf16, tag="hn", name="hnl")
                    ln_apply(h_t[:, t, :], mvh, rstdh, t, hn[:])
                    tp = psh.tile([P, P], bf16, space="PSUM", tag="tp",
                                  name="tp")
                    nc.tensor.transpose(out=tp[:], in_=hn[:],
                                        identity=id16[:])
                    hnT = ntmp.tile([P, P], bf16, tag="hnT2", name="hnT2")
                    nc.scalar.copy(out=hnT[:], in_=tp[:])
                    op_ = psh.tile([P, 4], fp32, space="PSUM", tag="mm",
                                   name="op")
                    nc.tensor.matmul(out=op_[:], lhsT=hnT[:], rhs=wl[:],
                                     start=True, stop=True)
                    ot = ntmp.tile([P, 4], fp32, tag="ot", name="ot")
                    nc.scalar.copy(out=ot[:], in_=op_[:])
                    nc.sync.dma_start(out=out_d[t * P:(t + 1) * P, :],
                                      in_=ot[:])

    nc.compile()
    return nc


def prep_inputs(x, edge_index, edge_attr,
                lin0_w, lin0_b,
                q_w, q_b, k_w, k_b, v_w, v_b, e_w, skip_w, skip_b,
                ln1_g, ln1_b, lins_w, lins_b, ln2_g, ln2_b,
                lins2_w, lins2_b, lnl_g, lnl_b, linl_w, linl_b):
    """Host-side sharding/sorting/folding."""
    x = np.asarray(x, np.float32)
    ei = np.asarray(edge_index, np.int64)
    ea = np.asarray(edge_attr, np.float32)
    src, dst = ei[0], ei[1]
    core = dst // NL
    slot = dst - core * NL

    def fold(W, bias, g, b):
        W = np.asarray(W, np.float64)
        Wf = W * np.asarray(g, np.float64)[None, :]
        cf = np.asarray(bias, np.float64) + W @ np.asarray(b, np.float64)
        return Wf.astype(np.float32), cf.astype(np.float32)

    wkvT = np.zeros((L, D, 2 * D), np.float32)
    wqsT = np.zeros((L, D, 2 * D), np.float32)
    w1T = np.zeros((L, D, D), np.float32)
    w2T = np.zeros((L, D, D), np.float32)
    ewdT = np.zeros((L, ED, 2 * D), np.float32)
    zero_bias = True
    for l in range(L):
        for (W, bias, dstT, half) in [
                (k_w[l], k_b[l], wkvT, 0), (v_w[l], v_b[l], wkvT, 1),
                (q_w[l], q_b[l], wqsT, 0), (skip_w[l], skip_b[l], wqsT, 1)]:
            Wf, cf = fold(W, bias, ln1_g[l], ln1_b[l])
            dstT[l, :, half * D:(half + 1) * D] = Wf.T
            zero_bias &= bool(np.abs(cf).max() == 0)
        w1T[l] = np.asarray(lins_w[l]).T
        zero_bias &= bool(np.abs(np.asarray(lins_b[l])).max() == 0)
        Wf, cf = fold(lins2_w[l], lins2_b[l], ln2_g[l], ln2_b[l])
        w2T[l] = Wf.T
        zero_bias &= bool(np.abs(cf).max() == 0)
        ewT = np.asarray(e_w[l]).T.astype(np.float32)   # [ED, D]
        ewdT[l, :, :D] = ewT
        ewdT[l, :, D:] = ewT
    Wl, cl = fold(linl_w, linl_b, lnl_g, lnl_b)
    wlT = np.zeros((D, 4), np.float32)
    wlT[:, :3] = Wl.T
    zero_bias &= bool(np.abs(cl).max() == 0)
    zero_bias &= bool(np.abs(np.asarray(lin0_b)).max() == 0)

    win = slot // 128
    counts = np.zeros((NCORES, NT), np.int64)
    np.add.at(counts, (core, win), 1)
    tiles_per_window = [max(1, int(math.ceil(counts[:, w].max() / 128)))
                        for w in range(NT)]
    tot_tiles = sum(tiles_per_window)
    tot_e = tot_tiles * 128

    in_maps = []
    order_all = np.lexsort((win, core))
    off = np.searchsorted(core[order_all], np.arange(NCORES + 1))
    kvrow_of = (src // NL) * NLP + (src % NL)

    for c in range(NCORES):
        oc = order_all[off[c]:off[c + 1]]
        wc = win[oc]
        woff = np.searchsorted(wc, np.arange(NT + 1))
        src_rows = np.zeros(tot_e, np.int16)
        dst_rows = np.zeros(tot_e, np.int16)
        onehot = np.zeros((tot_e, P), np.float32)
        ea_t = np.zeros((ED, tot_e), np.float32)
        base = 0
        for w in range(NT):
            ew_idx = oc[woff[w]:woff[w + 1]]
            k = len(ew_idx)
            sl = slice(base, base + k)
            src_rows[sl] = kvrow_of[ew_idx].astype(np.int16)
            dst_rows[sl] = slot[ew_idx].astype(np.int16)
            onehot[np.arange(base, base + k), slot[ew_idx] - w * 128] = 1.0
            ea_t[:, sl] = ea[ew_idx].T
            base += tiles_per_window[w] * 128
        assert base == tot_e

        def wrap(a):
            return np.tile(a.reshape(tot_e // 16, 16).T, (8, 1)).copy()

        xs = np.zeros((NLP, F), np.float32)
        xs[:NL] = x[c * NL:(c + 1) * NL]
        in_maps.append({
            "x_shard": xs,
            "idx_src": wrap(src_rows),
            "idx_dst": wrap(dst_rows),
            "onehot": onehot,
            "ea_t": ea_t,
            "wkvT": wkvT, "wqsT": wqsT,
            "w1T": w1T, "w2T": w2T, "ewdT": ewdT,
            "w0T": np.asarray(lin0_w).T.astype(np.float32),
            "ident": np.eye(P, dtype=np.float32),
            "wlT": wlT,
        })
    return in_maps, tiles_per_window, zero_bias


_CACHE = {}
LAST_RES = None


def kernel(**inputs):
    global LAST_RES
    import ml_dtypes
    in_maps, tiles_per_window, zero_bias = prep_inputs(**inputs)
    for m in in_maps:
        for k in ("onehot", "ea_t", "wkvT", "wqsT", "w1T", "w2T", "ewdT",
                  "wlT"):
            m[k] = m[k].astype(ml_dtypes.bfloat16)

    import os
    debug = bool(int(os.environ.get("K_DEBUG", "0")))
    key = (tuple(tiles_per_window), debug)
    if key not in _CACHE:
        _CACHE[key] = build(tiles_per_window, zero_bias, debug)
    nc = _CACHE[key]

    res = run_bass_kernel_spmd(nc, in_maps, core_ids=list(range(NCORES)))
    LAST_RES = res
    out = np.zeros((N, 3), np.float32)
    for c in range(NCORES):
        out[c * NL:(c + 1) * NL] = res.results[c]["out"][:NL, :3]
    return out
